# revision 10
# baseline (speedup 1.0000x reference)
"""Trainium2 Bass kernel v2 for the forward-attention LSA step
(nn_LSA_43404939494068). Measured 27.7us HW (wall-differenced hw-loop,
run-to-run spread ~27.7-29us) vs 61.1us for the v1 kernel; rel err 1.9e-3.

Contract: kernel(**inputs) takes FULL inputs, returns FULL [64,1,1024] f32.

Key structure:
  1. Mask-aware ragged packing: the output is exactly 0 where
     t >= phone_len[b] (att = scores*mask), so only sum(phone_len) = 33627
     of the 65536 (b,t) positions need ANY compute. Valid rows are packed
     into columns and split evenly across 8 cores -> ncol=4224 cols/core
     vs 8192 in v1: ~2x less DMA, matmul, tanh, everything. The program is
     compiled per ncol (value-specialized) inside kernel().
  2. Layout: partitions = a (4 blocks of 128), free dim = packed columns,
     processed in 512-col chunks (PSUM-bank sized). Per chunk, a-block j:
       z[a,col] = mcomb[:,a].T @ ls[:,col]      (PE mm1, K=62; odd j row-
                  packed at array rows 64+ via tile_position=(64,0) -> two
                  mm1s run concurrently: -5.8us measured)
       z       += eye @ encT[a,col]             (PE join, K=128) j=0,1
       xin      = z + encT                      (DVE add, 1x rate) j=2,3
       x        = tanh(z | xin)                 (ACT, 2 calls FD=1024)
       u       += v_j.T @ x_j                   (PE matvec M=1, 4 PSUM-
                                                 accumulated; v stationary)
     The x*v dot rides PE because the v1 DVE stt runs at 1x (594ns/512FD);
     DVE instead gets 2 of 4 enc-joins + the u PSUM->SBUF copy, balancing
     PE/ACT/DVE/DMA at ~15-18us each (cost model).
  3. The O(B*T) tail (sigmoid, alpha band, mask, normalize) runs on host
     numpy in fp64; device returns one u row per core. The reference's
     /sum(s) cancels in the final normalization and is skipped entirely.
  4. DMAs: everything on the SP HWDGE queue (ACT-queue issue costs ~1.1us
     of ACT SEQ per descriptor and delays the first tanh; SWDGE/gpsimd
     descriptor ucode overflows walrus' ISA length limit inside unrolled
     hw loops). Transfers complete in issue order, so chunk 0's enc slices
     go right after the two constants gating the first mm1, and each 2-chunk
     ls piece is woven in just before the enc group that needs it (a
     monolithic ls transfer in front of enc cost ~1us, measured).
PSUM budget (8 banks): zpe 2x2 + zdv 2x1 + u 1x2 = 8.
Failed experiments (HW-measured): col-tiled matvec via tile_position
(0,32j) is EXACT in an isolated program (ct_test.py) but corrupts rows in
kernel context - a neighboring full-width Ldweights (eye-join / next mm1,
interleaved by the tile scheduler) clobbers the col-tile weights before
their matmuls stream, and emission order cannot pin the scheduler; fused
single 4-bank z tile with u written into a consumed z bank over-serializes
(48.9us sim); ls/vb/out DMAs on the ACT queue cost +4.3us HW.
"""

import sys

import numpy as np

if "/opt/trn_rl_repo" not in sys.path:
    sys.path.insert(0, "/opt/trn_rl_repo")

import concourse.bass as bass
import concourse.tile as tile
from concourse import mybir
from concourse.bass_utils import run_bass_kernel_spmd

B, T, A = 64, 1024, 512
F, KW = 32, 31
PAD = (KW - 1) // 2
NCORES = 8
KC = 62                   # conv contraction = 2 channels * 31 taps
NAB = 4                   # a-blocks of 128
F32 = mybir.dt.float32
BF16 = mybir.dt.bfloat16

_MAX_WAITS = 1


def _split_sync_waits(nc):
    """walrus in this toolchain accepts at most one sync-wait per
    instruction; hoist excess waits onto NoOps inserted just before."""
    for fn in nc.m.functions:
        for blk in fn.blocks:
            new_list = []
            for inst in blk.instructions:
                si = inst.sync_info
                if si is not None and si.on_wait and len(si.on_wait) > _MAX_WAITS:
                    waits = list(si.on_wait)
                    extra, keep = waits[:-_MAX_WAITS], waits[-_MAX_WAITS:]
                    for i in range(0, len(extra), _MAX_WAITS):
                        nop = mybir.InstNoOp(
                            name=nc.get_next_instruction_name(),
                            sync_info=mybir.SyncInfo(
                                on_wait=extra[i:i + _MAX_WAITS], on_update=[]
                            ),
                            bass_nofuse=True,
                            engine=inst.engine,
                        )
                        nc.register_instruction(nop)
                        new_list.append(nop)
                    inst.sync_info = mybir.SyncInfo(
                        on_wait=keep, on_update=list(si.on_update)
                    )
                new_list.append(inst)
            blk.instructions[:] = new_list


def build_program(ncol: int, repeats: int = 1, hw_loop: bool = False,
                  unroll: int = 1, coltile_mv: bool = False,
                  pack_mm1: bool = True) -> bass.Bass:
    """ncol: padded packed columns per core (multiple of 128)."""
    CH = 512
    nfull, tail = divmod(ncol, CH)
    widths = [CH] * nfull + ([tail] if tail else [])

    nc = bass.Bass()

    enct_d = nc.declare_dram_parameter("enct", [NAB, 128, ncol], BF16, isOutput=False)
    ls_d = nc.declare_dram_parameter("ls", [126, ncol], BF16, isOutput=False)
    mcomb_d = nc.declare_dram_parameter("mcomb", [126, A], BF16, isOutput=False)
    eyeb_d = nc.declare_dram_parameter("eyeb", [128, 128], BF16, isOutput=False)
    vb_d = nc.declare_dram_parameter("vb", [128, NAB], BF16, isOutput=False)
    out_d = nc.declare_dram_parameter("out", [1, ncol], F32, isOutput=True)

    TANH = mybir.ActivationFunctionType.Tanh

    with tile.TileContext(nc) as tc:
        with (
            tc.tile_pool(name="const", bufs=1) as cpool,
            tc.tile_pool(name="xpe", bufs=3) as xpep,
            tc.tile_pool(name="xdv", bufs=3) as xdvp,
            tc.tile_pool(name="xin", bufs=3) as xinp,
            tc.tile_pool(name="zpe", bufs=2, space="PSUM") as zpep,
            tc.tile_pool(name="zdv", bufs=1, space="PSUM") as zdvp,
            tc.tile_pool(name="ups", bufs=2, space="PSUM") as upsp,
        ):
            # ---- constants (first-use order) ----
            mcomb_sb = cpool.tile([126, A], BF16, tag="mcomb")
            eyeb_sb = cpool.tile([128, 128], BF16, tag="eyeb")
            vb_sb = cpool.tile([128, NAB], BF16, tag="vb")

            ls_sb = cpool.tile([126, ncol], BF16, tag="ls")
            enct_sb = cpool.tile([128, NAB, ncol], BF16, tag="enct")
            u_sb = cpool.tile([1, ncol], F32, tag="usb")

            eps_sb = cpool.tile([128, 1], F32, tag="eps")
            nc.vector.memset(eps_sb[:], 1e-7)
            warm_sb = cpool.tile([128, 1], F32, tag="warm")
            nc.scalar.activation(out=warm_sb[:], in_=eps_sb[:], func=TANH)

            def body():
                # stage input DMAs chunk-major so compute streams right
                # behind the data; all on the SP HWDGE queue (see docstring).
                # Transfers complete strictly in issue order, so each ls
                # piece is woven in just before the enc group that needs it
                # (a monolithic ls transfer in front of enc delays the first
                # joins by ~3us), and chunk 0's enc slices go right after
                # the two constants that gate the first mm1.
                nc.sync.dma_start(out=mcomb_sb[:], in_=mcomb_d[:])
                nc.sync.dma_start(out=ls_sb[:, 0:CH], in_=ls_d[:, 0:CH])
                for j in range(NAB):
                    nc.sync.dma_start(out=enct_sb[:, j, 0:CH],
                                      in_=enct_d[j, :, 0:CH])
                    if j == 1:
                        nc.sync.dma_start(out=eyeb_sb[:], in_=eyeb_d[:])
                    if j == 3:
                        nc.sync.dma_start(out=vb_sb[:], in_=vb_d[:])
                c0 = CH
                while c0 < ncol:
                    W2 = min(2 * CH, ncol - c0)
                    nc.sync.dma_start(out=ls_sb[:, c0:c0 + W2],
                                      in_=ls_d[:, c0:c0 + W2])
                    for j in range(NAB):
                        nc.sync.dma_start(out=enct_sb[:, j, c0:c0 + W2],
                                          in_=enct_d[j, :, c0:c0 + W2])
                    c0 += W2

                # per-chunk compute, software-pipelined: the matvecs+copy of
                # chunk c-1 are issued between chunk c's mm1s and joins so
                # the in-order PE/DVE sequencers never idle during c's tanh.
                pending = []

                def emit_mv(prev):
                    xpe_p, xdv_p, csl_p, W_p = prev
                    u_ps = upsp.tile([128, CH], F32, tag="ups")
                    for j in range(NAB):
                        xt = (xpe_p[:, j, :W_p] if j < 2
                              else xdv_p[:, j - 2, :W_p])
                        nc.tensor.matmul(
                            u_ps[0:1, :W_p], vb_sb[:, j:j + 1], xt,
                            start=(j == 0), stop=(j == 3),
                            skip_group_check=True)
                    nc.vector.tensor_copy(out=u_sb[:, csl_p],
                                          in_=u_ps[0:1, :W_p])

                c0 = 0
                for ci, W in enumerate(widths):
                    csl = slice(c0, c0 + W)
                    zpe = zpep.tile([128, 2, CH], F32, tag="zpe")
                    zdv = zdvp.tile([128, 2, CH], F32, tag="zdv")
                    # mm1 for all 4 ablocks (j 0,1 -> zpe; 2,3 -> zdv);
                    # odd j row-packed at array rows 64+ (runs concurrent
                    # with the even j's mm1 when pack_mm1)
                    for j in range(NAB):
                        zt = zpe[:, j, :W] if j < 2 else zdv[:, j - 2, :W]
                        if pack_mm1 and (j % 2 == 1):
                            lhs = mcomb_sb[64:126, j * 128:(j + 1) * 128]
                            rhs = ls_sb[64:126, csl]
                            tp = (64, 0)
                        else:
                            lhs = mcomb_sb[0:62, j * 128:(j + 1) * 128]
                            rhs = ls_sb[0:62, csl]
                            tp = (0, 0) if pack_mm1 else None
                        nc.tensor.matmul(
                            zt, lhs, rhs,
                            start=True, stop=(j >= 2),
                            tile_position=tp, skip_group_check=True)
                    # PE eye-join for ablocks 0,1 (before the pipelined
                    # matvecs so the tanh input is ready as early as possible)
                    for j in range(2):
                        nc.tensor.matmul(
                            zpe[:, j, :W], eyeb_sb[:], enct_sb[:, j, csl],
                            start=False, stop=True, skip_group_check=True)
                    # DVE join for ablocks 2,3 (one call over both banks)
                    xin = xinp.tile([128, 2, CH], BF16, tag="xin")
                    nc.vector.tensor_add(xin[:, :, :W], zdv[:, :, :W],
                                         enct_sb[:, 2:4, csl])
                    # tanh
                    xpe = xpep.tile([128, 2, CH], BF16, tag="xpe")
                    nc.scalar.activation(out=xpe[:, :, :W], in_=zpe[:, :, :W],
                                         func=TANH)
                    xdv = xdvp.tile([128, 2, CH], BF16, tag="xdv")
                    nc.scalar.activation(out=xdv[:, :, :W], in_=xin[:, :, :W],
                                         func=TANH)
                    # pipelined matvecs of the previous chunk (PE + DVE copy)
                    if pending:
                        emit_mv(pending.pop())
                    pending.append((xpe, xdv, csl, W))
                    c0 += W

                emit_mv(pending.pop())
                nc.sync.dma_start(out=out_d[:], in_=u_sb[:])

            if hw_loop and repeats > 1:
                assert repeats % unroll == 0
                with tc.For_i(0, repeats // unroll, 1):
                    for _u in range(unroll):
                        body()
            else:
                for _rep in range(repeats):
                    body()

    _split_sync_waits(nc)
    return nc


def prep_inputs(inputs: dict):
    """Full inputs -> (per-core in_maps, ctx for assemble)."""
    import ml_dtypes

    enc = np.asarray(inputs["encoder_seq_proj"], np.float32)
    query = np.asarray(inputs["query"], np.float32)
    cum = np.asarray(inputs["cumulative"], np.float32)
    att = np.asarray(inputs["attention"], np.float32)
    alpha = np.asarray(inputs["alpha"], np.float32)
    conv_w = np.asarray(inputs["conv_w"], np.float32)
    L_w = np.asarray(inputs["L_w"], np.float32)
    L_b = np.asarray(inputs["L_b"], np.float32)
    W_w = np.asarray(inputs["W_w"], np.float32)
    W_b = np.asarray(inputs["W_b"], np.float32)
    v_w = np.asarray(inputs["v_w"], np.float32)
    phone_len = np.asarray(inputs["phone_len"]).astype(np.int64)

    # ---- ragged packing: rows (b, t) with t < phone_len[b] ----
    pl = np.clip(phone_len, 0, T)
    bidx = np.repeat(np.arange(B), pl)
    tidx = np.concatenate([np.arange(n) for n in pl]) if len(pl) else np.zeros(0, int)
    ntot = bidx.shape[0]
    ncol = -(-ntot // (NCORES * 128)) * 128      # per-core cols, 128-mult
    npad = NCORES * ncol - ntot
    bidx = np.concatenate([bidx, np.zeros(npad, np.int64)])
    tidx = np.concatenate([tidx, np.zeros(npad, np.int64)])

    # folded conv+projection weight: M[c*31+k, a] = sum_f conv_w[f,c,k]*L_w[a,f]
    mcomb = np.einsum("fck,af->cka", conv_w, L_w).reshape(KC, A)
    mcombd = np.zeros((126, A), np.float32)
    mcombd[0:KC] = mcomb
    mcombd[64:64 + KC] = mcomb
    mcombd = mcombd.astype(ml_dtypes.bfloat16)

    # processed query folded into enc (host weight algebra; tiny)
    pq = query @ W_w.T + (W_b + L_b)             # [B, A]
    encq = (enc + pq[:, None, :]).astype(ml_dtypes.bfloat16)  # [B,T,A]

    # location features, shifted taps: ls_g[ck, b, t]
    padc = np.zeros((B, T + 2 * PAD), np.float32)
    pada = np.zeros((B, T + 2 * PAD), np.float32)
    padc[:, PAD:PAD + T] = cum
    pada[:, PAD:PAD + T] = att
    ls_g = np.zeros((KC, B, T), np.float32)
    for k in range(KW):
        ls_g[k] = padc[:, k:k + T]
        ls_g[KW + k] = pada[:, k:k + T]
    ls_g = ls_g.astype(ml_dtypes.bfloat16)

    eyeb = np.eye(128, dtype=np.float32).astype(ml_dtypes.bfloat16)
    vb = np.ascontiguousarray(
        v_w[0].reshape(NAB, 128).T).astype(ml_dtypes.bfloat16)  # [128, 4]

    in_maps = []
    for c in range(NCORES):
        sl = slice(c * ncol, (c + 1) * ncol)
        bc, tc_ = bidx[sl], tidx[sl]
        # encT: [4, 128, ncol]
        ecols = encq[bc, tc_, :]                  # [ncol, 512] bf16
        enct = np.ascontiguousarray(
            ecols.T.reshape(NAB, 128, ncol))
        # ls: [126, ncol] with dup at rows 64..125
        lcols = ls_g[:, bc, tc_]                  # [62, ncol]
        lsd = np.zeros((126, ncol), ml_dtypes.bfloat16)
        lsd[0:KC] = lcols
        lsd[64:64 + KC] = lcols
        in_maps.append({
            "enct": enct,
            "ls": lsd,
            "mcomb": mcombd,
            "eyeb": eyeb,
            "vb": vb,
        })
    ctx = dict(ncol=ncol, ntot=ntot, bidx=bidx, tidx=tidx, pl=pl,
               alpha=alpha)
    return in_maps, ctx


def assemble_output(results: list[dict], ctx) -> np.ndarray:
    u = np.concatenate([np.asarray(r["out"], np.float64)[0]
                        for r in results])        # [8*ncol]
    u = u[:ctx["ntot"]]

    alpha = np.asarray(ctx["alpha"], np.float64)
    band = alpha.copy()
    band[:, 1:] += alpha[:, :-1]
    band[:, 2:] += alpha[:, :-2]
    band += 1e-7

    s = 1.0 / (1.0 + np.exp(-u))
    na = np.zeros((B, T), np.float64)
    na[ctx["bidx"][:ctx["ntot"]], ctx["tidx"][:ctx["ntot"]]] = s
    na *= band
    # re-zero anything past phone_len (padding rows wrote only real slots)
    na *= (np.arange(T)[None, :] < ctx["pl"][:, None])
    na /= na.sum(axis=1, keepdims=True)
    return na[:, None, :].astype(np.float32)


_CACHED = {}


def kernel(**inputs) -> np.ndarray:
    in_maps, ctx = prep_inputs(inputs)
    key = ctx["ncol"]
    if key not in _CACHED:
        _CACHED[key] = build_program(ncol=key, repeats=1)
    res = run_bass_kernel_spmd(_CACHED[key], in_maps, list(range(NCORES)))
    return assemble_output(res.results, ctx)
